# revision 1
# baseline (speedup 1.0000x reference)
"""Cross-attention kernel for Trainium2 (Bass/Tile), 8-core data-parallel, v8.

Reference computation (per batch element b):
    q = x @ Wq.T ; k = ctx @ Wk.T ; v = ctx @ Wv.T
    out = softmax((q @ k.T) * D**-0.5) @ v

Shapes: x [8, 2048, 1024], context [8, 2048, 1024], Wq/Wk/Wv [1024, 1024].

v3 notes (vs the 450us baseline):
 - Inputs are DMA'd "row-packed": each SBUF partition holds 2 consecutive
   DRAM rows, so descriptors are 8KB instead of 4KB (baseline DMA ran at
   210GB/s, descriptor-latency-bound; packed runs ~290GB/s). The index
   scrambling this causes (row = 256*pk + 2p + j) is harmless for pure
   contraction indices (t for ctx, e for Wq/Wk) as long as it is consistent
   on both sides of each matmul. x's s-scramble lands in output ROWS and is
   undone by a row-permuted output DMA access pattern; Wv's e-scramble
   would land in output COLUMNS, so it is undone early by strided drain
   copies after the PE transpose.
 - PE transposes are batched 8-blocks-to-a-PSUM-bank with a single [128,
   8*128] drain copy, alternating DVE / scalar so neither stalls the PE.
   (v2 tried the DMA xbar transpose engine: it emits 256B packets, ~10x
   slower than its cost model claims, and regressed to 549us.)
 - yT for the second s-half is computed between dots(h0) and the h0
   softmax, hiding the exp latency bubble and halving yt SBUF.
 - softmax-denominator column flips run in bf16 (fp32 PE transposes are 2x
   passes); flip PSUM comes from the transpose tag, row-sum PSUM from a
   matmul-tag tile's partition-0 row, so the matmul tag gets 5-deep
   rotation and transposes 3-deep in the 8 PSUM banks.
 - ~100 tiny dummy matmuls at t=0 keep the PE busy so the HAM clock gate
   is already warm (2.4GHz) when the first real matmuls issue.
"""

from contextlib import ExitStack

import numpy as np

B = 8
S = 2048  # query length
T = 2048  # key/value length
D = 1024  # model dim
P = 128
SCALE = float(D) ** -0.5

N_TB = T // P  # 16 key/value t-blocks
N_DT = D // P  # 8 contraction chunks
NPROJ = D // 512  # 2 x 512-wide chunks for [., 1024] outputs
RPACK = 2  # DRAM rows packed per SBUF partition
NPK_CTX = T // (P * RPACK)  # 8 packs for ctx / x
NPK_W = D // (P * RPACK)  # 4 packs for each weight
SH = S // 2  # s processed in two halves of 1024


def _emit_body(tc, x, ctxt, wq, wk, wv, out):
    import concourse.mybir as mybir
    from concourse.masks import make_identity

    fp32 = mybir.dt.float32
    bf16 = mybir.dt.bfloat16
    nc = tc.nc

    with ExitStack() as ctx:
        const = ctx.enter_context(tc.tile_pool(name="const", bufs=1))
        stage = ctx.enter_context(tc.tile_pool(name="stage", bufs=2))
        castp = ctx.enter_context(tc.tile_pool(name="castp", bufs=6))
        wnp = ctx.enter_context(tc.tile_pool(name="wnp", bufs=16))
        big8 = ctx.enter_context(tc.tile_pool(name="big8", bufs=4))
        ctp = ctx.enter_context(tc.tile_pool(name="ctp", bufs=16))
        wpp = ctx.enter_context(tc.tile_pool(name="wpp", bufs=8))
        vp = ctx.enter_context(tc.tile_pool(name="vp", bufs=16))
        ytp = ctx.enter_context(tc.tile_pool(name="ytp", bufs=8))
        smp = ctx.enter_context(tc.tile_pool(name="smp", bufs=2))
        sump = ctx.enter_context(tc.tile_pool(name="sump", bufs=8))

        ones_b = const.tile([P, 1], bf16, name="ones_b")
        nc.vector.memset(ones_b, 1.0)
        ident_1 = const.tile([1, 1], bf16, name="ident_1")
        nc.vector.memset(ident_1, 1.0)
        ident_b = const.tile([P, P], bf16, name="ident_b")
        make_identity(nc, ident_b)

        # wvg2[g] [128, 4, 1024]: Wv^T d-chunks 4g+cc on planes, e natural.
        # ctxT[tb] [128, 8, 128]: ctx^T t-block tb = 2*pk + j, d-chunk planes,
        #   with t = 256*pk + 2*f + j at free position f.
        # xtb[sb] [128, 4, 8, 128]: x^T for 512-col s-block sb, plane (jj, c).
        wvg2 = [
            big8.tile([P, 4, D], bf16, name=f"wvg{g}", tag="big") for g in range(2)
        ]
        ctxT = [
            ctp.tile([P, N_DT, P], bf16, name=f"ctxT{tb}", tag="ct")
            for tb in range(N_TB)
        ]
        v = [vp.tile([P, D], bf16, name=f"v{tb}", tag="v") for tb in range(N_TB)]

        with tc.tile_pool(name="psum", bufs=1, space="PSUM") as ps_pool:
            # ~100 tiny matmuls warm the HAM clock gate during the DMA head
            # (transpose-mode does not count as PE-busy for the HAM).
            warm_ps = ps_pool.tile([P, 512], fp32, name="warm", tag="mm", bufs=5)
            for w in range(100):
                nc.tensor.matmul(
                    warm_ps[0:1, 0:1], ones_b, ones_b, start=True, stop=True
                )

            # ------ input staging: DMA (row-packed) -> cast -> transpose ----
            drain_eng = [0]

            def load_pack(dram, npk, pk, nm):
                """DMA rows [pk*256, (pk+1)*256) as [128, 2, 1024] fp32 (8KB
                descriptors; partition p holds rows 256*pk + 2p + {0,1}), cast
                each row-plane to bf16. Returns the 2 bf16 [128, D] planes."""
                st = stage.tile([P, RPACK, D], fp32, name=f"st_{nm}", tag="stage")
                src = dram.rearrange("(k p j) d -> k p j d", k=npk, p=P, j=RPACK)
                nc.sync.dma_start(out=st, in_=src[pk])
                planes = []
                for j in range(RPACK):
                    bt = castp.tile([P, D], bf16, name=f"bf_{nm}_{j}", tag="cast")
                    nc.vector.tensor_copy(out=bt, in_=st[:, j, :])
                    planes.append(bt)
                return planes

            def drain_copy(dst, ps_src):
                if drain_eng[0] % 2 == 0:
                    nc.vector.tensor_copy(out=dst, in_=ps_src)
                else:
                    nc.scalar.copy(out=dst, in_=ps_src)
                drain_eng[0] += 1

            def transpose_plane(plane_bf, nm, drain_to):
                """PE-transpose the 8 128x128 blocks of a [128, D] bf16 tile
                into one PSUM bank; drain_to(ps) issues the drain copy(s)."""
                ps = ps_pool.tile(
                    [P, N_DT, P], bf16, name=f"tp_{nm}", tag="pt", bufs=3
                )
                for c in range(N_DT):
                    nc.tensor.transpose(
                        ps[:, c, :], plane_bf[:, c * P : (c + 1) * P], ident_b
                    )
                drain_to(ps)

            wv_planes = {}

            def wv_dma(pk):
                wv_planes[pk] = load_pack(wv, NPK_W, pk, f"wv{pk}")

            def wv_transp(pk):
                for j in range(RPACK):
                    # transpose col f holds e = 256*pk + 2f + j: unscramble
                    # into natural e order with strided drain copies.
                    def drain(ps, pk=pk, j=j):
                        for g in range(2):
                            dst = wvg2[g].rearrange(
                                "p c (q f j) -> p c q f j", q=NPK_W, f=P, j=RPACK
                            )[:, :, pk, :, j]
                            drain_copy(dst, ps[:, 4 * g : 4 * (g + 1), :])
                    transpose_plane(wv_planes[pk][j], f"wv{pk}_{j}", drain)

            ctx_planes = {}

            def ctx_dma(pk):
                ctx_planes[pk] = load_pack(ctxt, NPK_CTX, pk, f"c{pk}")

            def ctx_transp(pk):
                for j in range(RPACK):
                    def drain(ps, pk=pk, j=j):
                        drain_copy(ctxT[RPACK * pk + j], ps)
                    transpose_plane(ctx_planes[pk][j], f"c{pk}_{j}", drain)

            # DMA order: wv0 wv1 ctx0 ctx1 wv2 wv3 ctx2..7 (then wq/wk/x).
            # PE order is hand-scheduled so it never queues a wait on a pack
            # that lands later than work it already has in hand.
            wv_dma(0)
            wv_dma(1)
            ctx_dma(0)
            ctx_dma(1)
            wv_dma(2)
            wv_dma(3)
            ctx_dma(2)
            ctx_dma(3)
            wv_transp(0)
            wv_transp(1)
            ctx_transp(0)

            def mm_chunk(dst, dst_sl, stat_of, mov_of, n_acc, nm):
                """One [128, 512] output chunk accumulated over n_acc matmuls,
                drained to dst[dst_sl] (bf16) by the scalar engine."""
                ps = ps_pool.tile([P, 512], fp32, name=f"ps_{nm}", tag="mm", bufs=5)
                for a in range(n_acc):
                    nc.tensor.matmul(
                        ps, stat_of(a), mov_of(a), start=(a == 0), stop=(a == n_acc - 1)
                    )
                nc.scalar.copy(out=dst[:, dst_sl], in_=ps)

            def v_ne(tb, ne):
                mm_chunk(
                    v[tb],
                    slice(ne * 512, (ne + 1) * 512),
                    lambda c: ctxT[tb][:, c, :],
                    lambda c: wvg2[c // 4][:, c % 4, ne * 512 : (ne + 1) * 512],
                    N_DT,
                    f"v{tb}_{ne}",
                )

            # ne=0 chunks need only wv packs 0-1 (e < 512); fill the window
            # until wv packs 2-3 land with ne=0 work down the ctx packs.
            v_ne(0, 0)
            v_ne(1, 0)
            ctx_transp(1)
            v_ne(2, 0)
            v_ne(3, 0)
            wv_transp(2)
            wv_transp(3)
            v_ne(0, 1)
            v_ne(1, 1)
            ctx_dma(4)
            ctx_transp(2)
            v_ne(2, 1)
            v_ne(3, 1)
            ctx_dma(5)
            ctx_transp(3)
            v_ne(4, 0)
            v_ne(4, 1)
            v_ne(5, 0)
            v_ne(5, 1)
            v_ne(6, 0)
            v_ne(6, 1)
            v_ne(7, 0)
            v_ne(7, 1)
            for pk in range(4, NPK_CTX):
                if pk + 2 < NPK_CTX:
                    ctx_dma(pk + 2)
                ctx_transp(pk)
                for tb in (RPACK * pk, RPACK * pk + 1):
                    v_ne(tb, 0)
                    v_ne(tb, 1)

            # ---- Wq/Wk natural-layout planes (e scrambled consistently),
            #      W' = Wq^T @ Wk ----
            wqn, wkn = [], []
            for nm, dram, lst in (("wq", wq, wqn), ("wk", wk, wkn)):
                for pk in range(NPK_W):
                    st = stage.tile(
                        [P, RPACK, D], fp32, name=f"st_{nm}{pk}", tag="stage"
                    )
                    src = dram.rearrange("(k p j) d -> k p j d", k=NPK_W, p=P, j=RPACK)
                    nc.sync.dma_start(out=st, in_=src[pk])
                    for j in range(RPACK):
                        bt = wnp.tile([P, D], bf16, name=f"{nm}{pk}_{j}", tag="wn")
                        nc.vector.tensor_copy(out=bt, in_=st[:, j, :])
                        lst.append(bt)

            # ---- x^T packs interleaved with W' = Wq^T @ Wk: each W' chunk
            # pair (~3.4us) covers one x pack's DMA, so the PE never waits ----
            xtb = [
                big8.tile([P, 4, N_DT, P], bf16, name=f"xtb{sb}", tag="big")
                for sb in range(4)
            ]

            x_planes = {}

            def x_dma(pk):
                x_planes[pk] = load_pack(x, NPK_CTX, pk, f"x{pk}")

            def x_transp(pk):
                for j in range(RPACK):
                    sb, jj = pk // 2, 2 * (pk % 2) + j
                    def drain(ps, sb=sb, jj=jj):
                        drain_copy(xtb[sb][:, jj, :, :], ps)
                    transpose_plane(x_planes[pk][j], f"x{pk}_{j}", drain)

            wpb = [wpp.tile([P, D], bf16, name=f"wp{i}", tag="wp") for i in range(N_DT)]
            x_dma(0)
            for it in range(N_DT):
                x_dma(it + 1) if it + 1 < NPK_CTX else None
                for jn in range(NPROJ):
                    mm_chunk(
                        wpb[it],
                        slice(jn * 512, (jn + 1) * 512),
                        lambda e: wqn[e][:, it * P : (it + 1) * P],
                        lambda e: wkn[e][:, jn * 512 : (jn + 1) * 512],
                        N_DT,
                        f"wp{it}_{jn}",
                    )
                x_transp(it)

            # yt[jt] [128, 1024] holds yT d-chunk jt for one s-half; s columns
            # in scrambled order q = 512*sb + 128*jj + f <-> x row
            # 512*sb + 256*(jj//2) + 2f + (jj%2).
            def yt_half(h):
                tiles = [
                    ytp.tile([P, SH], bf16, name=f"yt{h}_{jt}", tag="yt")
                    for jt in range(N_DT)
                ]
                for sb in (2 * h, 2 * h + 1):
                    for jt in range(N_DT):
                        mm_chunk(
                            tiles[jt],
                            slice((sb % 2) * 512, (sb % 2 + 1) * 512),
                            lambda c: wpb[c][:, jt * P : (jt + 1) * P],
                            lambda c: xtb[sb][:, :, c, :],
                            N_DT,
                            f"yt{h}_{jt}_{sb}",
                        )
                return tiles

            yt0 = yt_half(0)

            # ---------- attention, two s-halves ----------
            def dots_exp(h, yth):
                """dots^T via ctx^T x yT contraction; exp straight out of PSUM
                on the scalar engine with the 1/32 scale folded in."""
                atT = []
                for tb in range(N_TB):
                    at = wnp.tile([P, SH], bf16, name=f"atT{h}_{tb}", tag="wn")
                    for ns in range(SH // 512):
                        ps = ps_pool.tile(
                            [P, 512], fp32, name=f"pd{h}_{tb}_{ns}", tag="mm", bufs=5
                        )
                        for c in range(N_DT):
                            nc.tensor.matmul(
                                ps,
                                ctxT[tb][:, c, :],
                                yth[c][:, ns * 512 : (ns + 1) * 512],
                                start=(c == 0),
                                stop=(c == N_DT - 1),
                            )
                        nc.scalar.activation(
                            out=at[:, ns * 512 : (ns + 1) * 512],
                            in_=ps,
                            func=mybir.ActivationFunctionType.Exp,
                            scale=SCALE,
                        )
                    atT.append(at)
                return atT

            def softmax_denoms(h, atT):
                """Column sums of attn^T: the (idle) DVE first adds tile
                pairs, halving the PE ones-matmul chain; sums are flipped
                into per-partition [128, 1] reciprocals (bf16 flips)."""
                pairs = []
                for u in range(N_TB // 2):
                    s = sump.tile([P, SH], bf16, name=f"as{h}_{u}", tag="as")
                    nc.vector.tensor_tensor(
                        s, atT[2 * u], atT[2 * u + 1], mybir.AluOpType.add
                    )
                    pairs.append(s)
                srows = []
                for ns in range(SH // 512):
                    pst = ps_pool.tile(
                        [P, 512], fp32, name=f"pss{h}_{ns}", tag="mm", bufs=5
                    )
                    pss = pst[0:1, :]
                    for u in range(N_TB // 2):
                        nc.tensor.matmul(
                            pss,
                            ones_b,
                            pairs[u][:, ns * 512 : (ns + 1) * 512],
                            start=(u == 0),
                            stop=(u == N_TB // 2 - 1),
                        )
                    srow = smp.tile([1, 512], bf16, name=f"srow{h}_{ns}", tag="srow")
                    nc.vector.tensor_copy(out=srow, in_=pss)
                    srows.append(srow)
                recips = []
                for sl in range(8):
                    ns, off = sl // 4, (sl % 4) * P
                    pct = ps_pool.tile(
                        [P, N_DT, P], bf16, name=f"psc{h}_{sl}", tag="pt", bufs=3
                    )
                    psc = pct[:, 0, 0:1]
                    nc.tensor.transpose(psc, srows[ns][0:1, off : off + P], ident_1)
                    recip = smp.tile(
                        [P, 1], fp32, name=f"rc{h}_{sl}", tag="recip", bufs=8
                    )
                    nc.vector.reciprocal(out=recip, in_=psc)
                    recips.append(recip)
                return recips

            # out row for block sl = 4*sb + jj at partition f:
            #   512*sb + 256*(jj//2) + 2f + (jj%2)
            out_r = out.rearrange("(sb a f b) d -> sb a f b d", sb=4, a=2, f=P, b=RPACK)

            def attnv(h, atT, recips):
                for spair in range(4):
                    o = stage.tile(
                        [P, RPACK, D], fp32, name=f"o{h}_{spair}", tag="stage"
                    )
                    gsl = h * 8 + 2 * spair
                    split_last = h == 1 and spair == 3
                    for b in range(RPACK):
                        sl = 2 * spair + b
                        for ne in range(NPROJ):
                            ps = ps_pool.tile(
                                [P, 512], fp32, name=f"pav{h}_{sl}_{ne}", tag="mm",
                                bufs=5,
                            )
                            for tb in range(N_TB):
                                nc.tensor.matmul(
                                    ps,
                                    atT[tb][:, sl * P : (sl + 1) * P],
                                    v[tb][:, ne * 512 : (ne + 1) * 512],
                                    start=(tb == 0),
                                    stop=(tb == N_TB - 1),
                                )
                            nc.scalar.mul(
                                out=o[:, b, ne * 512 : (ne + 1) * 512],
                                in_=ps,
                                mul=recips[sl],
                            )
                        if split_last:
                            # shave the exit tail: ship each 256KB quarter as
                            # soon as its mul lands instead of one 1MB DMA
                            for ne in range(NPROJ):
                                nc.sync.dma_start(
                                    out=out_r[
                                        gsl // 4, (gsl % 4) // 2, :, b,
                                        ne * 512 : (ne + 1) * 512,
                                    ],
                                    in_=o[:, b, ne * 512 : (ne + 1) * 512],
                                )
                    if not split_last:
                        nc.sync.dma_start(
                            out=out_r[gsl // 4, (gsl % 4) // 2], in_=o
                        )

            atT0 = dots_exp(0, yt0)
            yt1 = yt_half(1)  # fills the PE while the scalar engine exps h0
            rec0 = softmax_denoms(0, atT0)
            attnv(0, atT0, rec0)
            atT1 = dots_exp(1, yt1)
            rec1 = softmax_denoms(1, atT1)
            attnv(1, atT1, rec1)


def build_nc():
    import concourse.mybir as mybir
    import concourse.tile as tile
    from concourse import bacc

    fp32 = mybir.dt.float32
    nc = bacc.Bacc("TRN2", target_bir_lowering=False, debug=False)
    x = nc.dram_tensor("x", [S, D], fp32, kind="ExternalInput").ap()
    ctxt = nc.dram_tensor("context", [T, D], fp32, kind="ExternalInput").ap()
    wq = nc.dram_tensor("Wq", [D, D], fp32, kind="ExternalInput").ap()
    wk = nc.dram_tensor("Wk", [D, D], fp32, kind="ExternalInput").ap()
    wv = nc.dram_tensor("Wv", [D, D], fp32, kind="ExternalInput").ap()
    out = nc.dram_tensor("out", [S, D], fp32, kind="ExternalOutput").ap()
    with tile.TileContext(nc) as tc:
        _emit_body(tc, x, ctxt, wq, wk, wv, out)
    nc.compile()
    return nc


_CACHED_NC = None


def kernel(**inputs):
    global _CACHED_NC
    from concourse.bass_utils import run_bass_kernel_spmd

    x = np.ascontiguousarray(np.asarray(inputs["x"], dtype=np.float32))
    ctxt = np.ascontiguousarray(np.asarray(inputs["context"], dtype=np.float32))
    wq = np.ascontiguousarray(np.asarray(inputs["Wq"], dtype=np.float32))
    wk = np.ascontiguousarray(np.asarray(inputs["Wk"], dtype=np.float32))
    wv = np.ascontiguousarray(np.asarray(inputs["Wv"], dtype=np.float32))

    if _CACHED_NC is None:
        _CACHED_NC = build_nc()
    nc = _CACHED_NC

    in_maps = [
        {"x": x[b], "context": ctxt[b], "Wq": wq, "Wk": wk, "Wv": wv}
        for b in range(B)
    ]
    res = run_bass_kernel_spmd(nc, in_maps, core_ids=list(range(B)))
    return np.stack([res.results[b]["out"] for b in range(B)], axis=0)

